# revision 1
# baseline (speedup 1.0000x reference)
"""DHASPI level-loss kernel for 8 Trainium2 NeuronCores.

Data-parallel over the fused B*C row axis: each of the 8 cores processes 64
rows of x_env and 64 rows of y_env (the x rows sit in SBUF partitions 0-63,
the y rows in partitions 64-127, so every DMA is a full 128-partition
transfer). Per row the kernel computes the gated LUFS loudness; the final
relu-diff scalar sum over the 512 rows is done on the host from the 8 tiny
[128, 1] per-core outputs.

Math notes:
- Frame energies (9600-sample windows, shift 2880) are built from 960-sample
  block sums: gcd(9600, 2880) = 960, frame f = blocks 3f..3f+9.
- Per 9600-sample chunk: the Scalar engine squares it (activation Square),
  the Vector engine then block-reduces [128, 10, 960] -> [128, 10]. The two
  engines pipeline across chunks; both stay under the HBM roofline, so the
  kernel is DMA-bound as intended for this memory-regime problem.
- All dB-domain gating comparisons are done in the energy domain via the
  monotone map el = -0.691 + 10*log10(z + eps):
    el > -70           <=>  z > 10**(-6.9309) - eps
    el > gamma_r       <=>  z > 0.1*(z_ave_a + eps) - eps
  so the only transcendental on device is one Ln per row at the end.

Written in raw Bass (explicit semaphores, double-buffered DMA + squares):
the Tile framework's kernel-tail drain emits multi-wait instructions this
walrus build rejects, and the fused accumulate features (tensor_tensor_reduce,
accum_out) are dropped by its codegen — so plain BIR ops with manual sync.
"""

import math

import numpy as np

import concourse.bass as bass
from concourse import mybir
from concourse.bass_utils import run_bass_kernel_spmd

# Problem constants (hardcoded from the spec; kernel.py must be self-contained)
B, C, T = 16, 32, 192000
N_CORES = 8
ROWS = B * C  # 512
RPC = ROWS // N_CORES  # 64 rows per core per tensor

FRAME = 9600
SHIFT = 2880
BLK = 960  # gcd(FRAME, SHIFT)
NBLK = T // BLK  # 200 block sums per row
NFRM = (T - FRAME) // SHIFT + 1  # 64 frames per row
CHUNK = 9600  # chunk size (4.9 MB per 128-row transfer)
# The last main chunk is split into small tail chunks so the final
# square+reduce after the last DMA is short (~4 us instead of ~19 us).
CHUNKS = [CHUNK] * (T // CHUNK - 1) + [1920] * (CHUNK // 1920)
NBUF = 2  # input chunk buffers (and squared-chunk buffers)

EPS = 1e-8
ALPHA = 1e-4
GAMMA_A = -70.0
# z-domain threshold equivalent to el > GAMMA_A
TA = float(10.0 ** ((GAMMA_A + 0.691) / 10.0) - EPS)
# relative threshold: z > 0.1*(z_ave_a + EPS) - EPS = 0.1*z_ave_a + TR_OFF
TR_OFF = float(0.1 * EPS - EPS)
LN10_INV10 = float(10.0 / math.log(10.0))
INV_FRAME = float(1.0 / FRAME)

F32 = mybir.dt.float32


def _overlapped_frames_view(bs_ap):
    """[128, NFRM, 10] view of the block-sum tile: frame f = blocks 3f..3f+9."""
    base = bs_ap[:, 0:1]
    return type(base)(
        tensor=base.tensor,
        offset=base.offset,
        ap=[list(base.ap[0]), [3, NFRM], [1, FRAME // BLK]],
    )


def _build_program(debug_stop: str | None = None) -> bass.Bass:
    """debug_stop: None=full kernel, 'loop'=skip epilogue (dev-only knob)."""
    nc = bass.Bass("TRN2", target_bir_lowering=False, debug=False)
    AF = mybir.ActivationFunctionType
    ALU = mybir.AluOpType
    AX = mybir.AxisListType

    xy = nc.dram_tensor("xy", [128, T], F32, kind="ExternalInput").ap()
    out = nc.dram_tensor("lufs", [128, 1], F32, kind="ExternalOutput").ap()

    # SBUF working set, per partition: 2*37.5KB input + 2*37.5KB squared
    # + ~3KB small tiles = ~153KB of the 192KB budget.
    xt = [nc.alloc_sbuf_tensor(f"xt{i}", [128, CHUNK], F32).ap() for i in range(NBUF)]
    sq = [nc.alloc_sbuf_tensor(f"sq{i}", [128, CHUNK], F32).ap() for i in range(NBUF)]
    bs = nc.alloc_sbuf_tensor("bs", [128, NBLK], F32).ap()
    zsum = nc.alloc_sbuf_tensor("zsum", [128, NFRM], F32).ap()
    z = nc.alloc_sbuf_tensor("z", [128, NFRM], F32).ap()
    ga = nc.alloc_sbuf_tensor("ga", [128, NFRM], F32).ap()
    ma = nc.alloc_sbuf_tensor("ma", [128, NFRM], F32).ap()
    gar = nc.alloc_sbuf_tensor("gar", [128, NFRM], F32).ap()
    junk = nc.alloc_sbuf_tensor("junk", [128, NFRM], F32).ap()
    sc = nc.alloc_sbuf_tensor("sc", [128, 12], F32).ap()  # per-row scalars
    eps_t = nc.alloc_sbuf_tensor("eps_t", [128, 1], F32).ap()

    numa = sc[:, 0:1]
    dena = sc[:, 1:2]
    rca = sc[:, 2:3]
    zavea = sc[:, 3:4]
    thr = sc[:, 4:5]
    denar = sc[:, 5:6]
    numar = sc[:, 6:7]
    rcar = sc[:, 7:8]
    zavear = sc[:, 8:9]
    lnz = sc[:, 9:10]
    lufs_t = sc[:, 10:11]

    with (
        nc.Block() as block,
        nc.semaphore("dma_sem0") as dma_sem0,
        nc.semaphore("dma_sem1") as dma_sem1,
        nc.semaphore("dma_out_sem") as dma_out_sem,
        nc.semaphore("act_sem") as act_sem,
        nc.semaphore("dve_sem") as dve_sem,
    ):
        # One DMA-completion sem per buffer slot: a shared cumulative sem is
        # unsafe with >1 DMA in flight (a later DMA's per-engine increments
        # can reach the threshold while an earlier DMA is still draining).
        dma_sems = [dma_sem0, dma_sem1]

        n_chunks = len(CHUNKS)
        offs = [sum(CHUNKS[:i]) for i in range(n_chunks)]

        @block.sync
        def _(sync):
            for c, (off, size) in enumerate(zip(offs, CHUNKS)):
                if c >= NBUF:
                    # input slot c%NBUF is free once ACT has squared chunk c-NBUF
                    sync.wait_ge(act_sem, c - NBUF + 1)
                sync.dma_start(
                    out=xt[c % NBUF][:, 0:size], in_=xy[:, off : off + size]
                ).then_inc(dma_sems[c % NBUF], 16)
            # final output DMA after the epilogue chain finishes
            sync.wait_ge(dve_sem, n_chunks + 2)
            sync.dma_start(out=out, in_=lufs_t).then_inc(dma_out_sem, 16)
            sync.wait_ge(dma_out_sem, 16)

        @block.scalar
        def _(scalar):
            for c, size in enumerate(CHUNKS):
                scalar.wait_ge(dma_sems[c % NBUF], (c // NBUF + 1) * 16)
                if c >= NBUF:
                    # sq slot c%NBUF is free once DVE has reduced chunk c-NBUF
                    scalar.wait_ge(dve_sem, c - NBUF + 1)
                scalar.activation(
                    sq[c % NBUF][:, 0:size], xt[c % NBUF][:, 0:size], AF.Square
                )
                # flush the pipe before signalling so DVE reads landed data
                scalar.drain().then_inc(act_sem, 1)
            # epilogue: ln(z_ave_ar + EPS) once DVE has produced z_ave_ar
            scalar.wait_ge(dve_sem, n_chunks + 1)
            scalar.activation(lnz, zavear, AF.Ln, bias=eps_t)
            scalar.drain().then_inc(act_sem, 1)

        @block.vector
        def _(vector):
            ALU_ = ALU
            vector.memset(eps_t, EPS)
            for c, (off, size) in enumerate(zip(offs, CHUNKS)):
                vector.wait_ge(act_sem, c + 1)
                sqv = sq[c % NBUF][:, 0:size].rearrange("p (n b) -> p n b", b=BLK)
                vector.reduce_sum(
                    bs[:, off // BLK : (off + size) // BLK], sqv, axis=AX.X
                ).then_inc(dve_sem, 1)

            # ---- epilogue (all [128, NFRM] or [128, 1] ops) ----
            # Raw-bass DVE instructions race on same-engine RAW (no implicit
            # pipeline flush between instructions on this HW), so drain()
            # between every dependent pair. ~12 drains ≈ a few µs, off the
            # critical path.
            if debug_stop == "loop":
                vector.memset(zavear, 1.0)
                vector.drain().then_inc(dve_sem, 1)
                vector.wait_ge(act_sem, n_chunks + 1)
                vector.tensor_scalar_mul(lufs_t, lnz, 1.0)
                vector.drain().then_inc(dve_sem, 1)
                return
            # z[f] = (sum of blocks 3f..3f+9) / FRAME
            vector.drain()
            vector.reduce_sum(zsum[:, :], _overlapped_frames_view(bs), axis=AX.X)
            vector.drain()
            vector.tensor_scalar_mul(z[:, :], zsum[:, :], INV_FRAME)
            vector.drain()
            # absolute gating: ma = (z > TA), ga = ma * z
            vector.scalar_tensor_tensor(
                out=ga[:, :], in0=z[:, :], scalar=TA, in1=z[:, :],
                op0=ALU_.is_gt, op1=ALU_.mult,
            )
            vector.tensor_scalar(ma[:, :], z[:, :], TA, None, op0=ALU_.is_gt)
            vector.drain()
            vector.reduce_sum(numa, ga[:, :], axis=AX.X)
            vector.reduce_sum(dena, ma[:, :], axis=AX.X)
            vector.drain()
            # z_ave_a = numa / (dena + EPS); relative threshold
            vector.tensor_scalar_add(dena, dena, EPS)
            vector.drain()
            vector.reciprocal(rca, dena)
            vector.drain()
            vector.tensor_tensor(zavea, numa, rca, op=ALU_.mult)
            vector.drain()
            vector.tensor_scalar(thr, zavea, 0.1, TR_OFF, op0=ALU_.mult, op1=ALU_.add)
            vector.drain()
            # relative gating: gar = (z > thr) * ma; numar = sum(z * gar)
            vector.scalar_tensor_tensor(
                out=gar[:, :], in0=z[:, :], scalar=thr, in1=ma[:, :],
                op0=ALU_.is_gt, op1=ALU_.mult,
            )
            vector.drain()
            # z*gar = (z > thr) * ga, so reuse ga instead of a fresh multiply
            vector.scalar_tensor_tensor(
                out=junk[:, :], in0=z[:, :], scalar=thr, in1=ga[:, :],
                op0=ALU_.is_gt, op1=ALU_.mult,
            )
            vector.reduce_sum(denar, gar[:, :], axis=AX.X)
            vector.drain()
            vector.reduce_sum(numar, junk[:, :], axis=AX.X)
            vector.drain()
            # z_ave_ar = numar / (denar + EPS)
            vector.tensor_scalar_add(denar, denar, EPS)
            vector.drain()
            vector.reciprocal(rcar, denar)
            vector.drain()
            vector.tensor_tensor(zavear, numar, rcar, op=ALU_.mult)
            vector.drain().then_inc(dve_sem, 1)
            # lufs = -0.691 + (10/ln10) * ln(z_ave_ar + EPS); ln from ACT
            vector.wait_ge(act_sem, n_chunks + 1)
            vector.tensor_scalar(
                lufs_t, lnz, LN10_INV10, -0.691, op0=ALU_.mult, op1=ALU_.add
            )
            vector.drain().then_inc(dve_sem, 1)

    return nc


def make_in_maps(x_env: np.ndarray, y_env: np.ndarray) -> list[dict[str, np.ndarray]]:
    x = np.asarray(x_env, dtype=np.float32).reshape(ROWS, T)
    y = np.asarray(y_env, dtype=np.float32).reshape(ROWS, T)
    in_maps = []
    for i in range(N_CORES):
        shard = np.concatenate(
            [x[i * RPC : (i + 1) * RPC], y[i * RPC : (i + 1) * RPC]], axis=0
        )
        in_maps.append({"xy": np.ascontiguousarray(shard)})
    return in_maps


def finish(per_core_lufs: list[np.ndarray]) -> np.ndarray:
    total = 0.0
    for lf in per_core_lufs:
        lf = np.asarray(lf).reshape(128).astype(np.float64)
        total += np.maximum(lf[RPC:] - lf[:RPC], 0.0).sum()
    return np.array(ALPHA * total, dtype=np.float32)


def kernel(x_env: np.ndarray, y_env: np.ndarray) -> np.ndarray:
    nc = _build_program()
    in_maps = make_in_maps(x_env, y_env)
    res = run_bass_kernel_spmd(nc, in_maps, core_ids=list(range(N_CORES)))
    return finish([res.results[i]["lufs"] for i in range(N_CORES)])



# revision 2
# speedup vs baseline: 2.5482x; 2.5482x over previous
"""DHASPI level-loss kernel for 8 Trainium2 NeuronCores.

Data-parallel over the fused B*C row axis: each of the 8 cores processes 64
rows of x_env and 64 rows of y_env (x rows in SBUF partitions 0-63, y rows in
64-127, so every DMA is a full 128-partition transfer).

Dataflow per core (per 9600-sample chunk, 20 chunks):
- gpsimd (SWDGE) cast-DMA streams the chunk HBM f32 -> SBUF fp8(e4m3).
  fp8 is safe here: the loss compares 10*log10 of ~192000-sample mean-square
  energies between x and y; e4m3's quantization bias on E[x^2] (~+0.13%) is
  identical for x and y rows so it cancels in the loudness difference, and
  the per-row random part is ~0.02% (≈0.001 dB) against a 2e-2 tolerance.
- The 10 blocks of 960 samples are split between the two elementwise
  engines, both of which produce f32 block energy sums directly:
    ACT:  activation(Square, accum_out=bs column)  (~1.17 us/block)
    DVE:  scalar_tensor_tensor(x*1.0*x, accum_out) (~1.06 us/block)
  The per-chunk split follows the pattern [5,5,5,4] ACT blocks so both
  engines carry ~111 us of work per core and neither is the long pole.
- Epilogue (DVE, f32): frame energies via an overlapped [128, 64, 10] view
  of the block sums (frame f = blocks 3f..3f+9), absolute + relative gating
  entirely in the (unscaled) energy domain, then outputs per-row
  numar' = sum(z'*gate) and denar = sum(gate). The host finishes with
  z_ave_ar = numar'/FRAME/(denar+EPS), the single log10, and the relu-diff
  scalar sum -- so no activation-table switch and no Ln on device.

Gating is done on z' = FRAME*z (unscaled frame sums) via the monotone map
el = -0.691 + 10*log10(z + eps):
    el > -70      <=>  z' > FRAME*(10**(-6.9309) - EPS)
    el > gamma_r  <=>  z' > 0.1*z_ave_a' + FRAME*(0.1*EPS - EPS)

Raw Bass (explicit semaphores): the Tile framework's kernel-tail drain emits
multi-wait instructions this walrus build rejects. tensor_tensor_reduce is
rejected by walrus codegen (visitInstISA), but activation/stt accum_out both
compile and are bit-exact (verified on HW against numpy).
"""

import math

import numpy as np

import concourse.bass as bass
from concourse import mybir
from concourse.bass_utils import run_bass_kernel_spmd

# Problem constants (hardcoded from the spec; kernel.py must be self-contained)
B, C, T = 16, 32, 192000
N_CORES = 8
ROWS = B * C  # 512
RPC = ROWS // N_CORES  # 64 rows per core per tensor

FRAME = 9600
SHIFT = 2880
BLK = 960  # gcd(FRAME, SHIFT)
NBLK = T // BLK  # 200 block sums per row
NFRM = (T - FRAME) // SHIFT + 1  # 64 frames per row
CHUNK = 9600  # 10 blocks per chunk
N_CHUNKS = T // CHUNK  # 20
# ACT blocks per chunk; remainder goes to DVE. [5,5,5,4] balances the two
# engines' totals (ACT ~1.17 us/block incl. accumulator read, DVE ~1.06).
ACT_BLOCKS = [[5, 5, 5, 4][c % 4] for c in range(N_CHUNKS)]
NBUF = 4  # input chunk slots

EPS = 1e-8
ALPHA = 1e-4
GAMMA_A = -70.0
# z'-domain (unscaled frame-sum) thresholds
TA = float((10.0 ** ((GAMMA_A + 0.691) / 10.0) - EPS) * FRAME)
TR_OFF = float((0.1 * EPS - EPS) * FRAME)

F32 = mybir.dt.float32
BF16 = mybir.dt.bfloat16
FP8 = mybir.dt.float8e4


def _overlapped_frames_view(bs_ap):
    """[128, NFRM, 10] view of the block-sum tile: frame f = blocks 3f..3f+9."""
    base = bs_ap[:, 0:1]
    return type(base)(
        tensor=base.tensor,
        offset=base.offset,
        ap=[list(base.ap[0]), [3, NFRM], [1, FRAME // BLK]],
    )


def _build_program() -> bass.Bass:
    nc = bass.Bass("TRN2", target_bir_lowering=False, debug=False)
    AF = mybir.ActivationFunctionType
    ALU = mybir.AluOpType
    AX = mybir.AxisListType

    xy = nc.dram_tensor("xy", [128, T], F32, kind="ExternalInput").ap()
    out = nc.dram_tensor("nd", [128, 2], F32, kind="ExternalOutput").ap()

    # SBUF per partition: 4 * 9.4KB fp8 chunk slots + ~8KB small tiles.
    xt = [nc.alloc_sbuf_tensor(f"xt{i}", [128, CHUNK], FP8).ap() for i in range(NBUF)]
    junk_a = nc.alloc_sbuf_tensor("junk_a", [128, BLK], BF16).ap()
    junk_d = nc.alloc_sbuf_tensor("junk_d", [128, BLK], BF16).ap()
    bs = nc.alloc_sbuf_tensor("bs", [128, NBLK], F32).ap()
    zs = nc.alloc_sbuf_tensor("zs", [128, NFRM], F32).ap()
    ga = nc.alloc_sbuf_tensor("ga", [128, NFRM], F32).ap()
    ma = nc.alloc_sbuf_tensor("ma", [128, NFRM], F32).ap()
    gar = nc.alloc_sbuf_tensor("gar", [128, NFRM], F32).ap()
    junk2 = nc.alloc_sbuf_tensor("junk2", [128, NFRM], F32).ap()
    sc = nc.alloc_sbuf_tensor("sc", [128, 8], F32).ap()  # per-row scalars

    numar = sc[:, 0:1]
    denar = sc[:, 1:2]
    numa = sc[:, 2:3]
    dena = sc[:, 3:4]
    rca = sc[:, 4:5]
    zavea = sc[:, 5:6]
    thr = sc[:, 6:7]

    with (
        nc.Block() as block,
        nc.semaphore("dma_sem0") as dma_sem0,
        nc.semaphore("dma_sem1") as dma_sem1,
        nc.semaphore("dma_sem2") as dma_sem2,
        nc.semaphore("dma_sem3") as dma_sem3,
        nc.semaphore("act_sem") as act_sem,
        nc.semaphore("dve_sem") as dve_sem,
        nc.semaphore("out_sem") as out_sem,
    ):
        # One DMA-completion sem per buffer slot: a shared cumulative sem is
        # unsafe with >1 DMA in flight (per-engine increments of a later DMA
        # can reach the threshold while an earlier one is still draining).
        dma_sems = [dma_sem0, dma_sem1, dma_sem2, dma_sem3]

        @block.gpsimd
        def _(g):
            for c in range(N_CHUNKS):
                if c >= NBUF:
                    # slot c%NBUF is free once both engines finished c-NBUF
                    g.wait_ge(act_sem, c - NBUF + 1)
                    g.wait_ge(dve_sem, c - NBUF + 1)
                g.dma_start(
                    out=xt[c % NBUF], in_=xy[:, c * CHUNK : (c + 1) * CHUNK]
                ).then_inc(dma_sems[c % NBUF], 16)

        @block.scalar
        def _(scalar):
            for c in range(N_CHUNKS):
                scalar.wait_ge(dma_sems[c % NBUF], (c // NBUF + 1) * 16)
                for b in range(ACT_BLOCKS[c]):
                    scalar.activation(
                        junk_a,
                        xt[c % NBUF][:, b * BLK : (b + 1) * BLK],
                        AF.Square,
                        accum_out=bs[:, c * 10 + b : c * 10 + b + 1],
                    )
                # flush writes before signalling readers on other queues
                scalar.drain().then_inc(act_sem, 1)

        @block.vector
        def _(v):
            ALU_ = ALU
            for c in range(N_CHUNKS):
                v.wait_ge(dma_sems[c % NBUF], (c // NBUF + 1) * 16)
                for b in range(ACT_BLOCKS[c], 10):
                    blk_ap = xt[c % NBUF][:, b * BLK : (b + 1) * BLK]
                    v.scalar_tensor_tensor(
                        out=junk_d,
                        in0=blk_ap,
                        scalar=1.0,
                        in1=blk_ap,
                        op0=ALU_.mult,
                        op1=ALU_.mult,
                        accum_out=bs[:, c * 10 + b : c * 10 + b + 1],
                    )
                v.drain().then_inc(dve_sem, 1)

            # ---- epilogue (f32, [128, NFRM] or [128, 1] ops) ----
            # Raw-bass DVE instructions race on same-engine RAW, so drain()
            # between every dependent pair (~10 drains, off the main loop).
            v.wait_ge(act_sem, N_CHUNKS)
            v.drain()
            # z'[f] = sum of blocks 3f..3f+9 (unscaled frame energy)
            v.reduce_sum(zs[:, :], _overlapped_frames_view(bs), axis=AX.X)
            v.drain()
            # absolute gating: ma = (z' > TA'), ga = ma * z'
            v.scalar_tensor_tensor(
                out=ga[:, :], in0=zs[:, :], scalar=TA, in1=zs[:, :],
                op0=ALU_.is_gt, op1=ALU_.mult,
            )
            v.tensor_scalar(ma[:, :], zs[:, :], TA, None, op0=ALU_.is_gt)
            v.drain()
            v.reduce_sum(numa, ga[:, :], axis=AX.X)
            v.reduce_sum(dena, ma[:, :], axis=AX.X)
            v.drain()
            # z_ave_a' = numa / (dena + EPS); relative threshold
            v.tensor_scalar_add(dena, dena, EPS)
            v.drain()
            v.reciprocal(rca, dena)
            v.drain()
            v.tensor_tensor(zavea, numa, rca, op=ALU_.mult)
            v.drain()
            v.tensor_scalar(thr, zavea, 0.1, TR_OFF, op0=ALU_.mult, op1=ALU_.add)
            v.drain()
            # relative gating: gar = (z' > thr) * ma; numar = sum(z' * gar)
            v.scalar_tensor_tensor(
                out=gar[:, :], in0=zs[:, :], scalar=thr, in1=ma[:, :],
                op0=ALU_.is_gt, op1=ALU_.mult,
            )
            v.drain()
            # z'*gar = (z' > thr) * ga, so reuse ga instead of a fresh multiply
            v.scalar_tensor_tensor(
                out=junk2[:, :], in0=zs[:, :], scalar=thr, in1=ga[:, :],
                op0=ALU_.is_gt, op1=ALU_.mult,
            )
            v.reduce_sum(denar, gar[:, :], axis=AX.X)
            v.drain()
            v.reduce_sum(numar, junk2[:, :], axis=AX.X)
            v.drain().then_inc(dve_sem, 1)

        @block.sync
        def _(sync):
            sync.wait_ge(dve_sem, N_CHUNKS + 1)
            sync.dma_start(out=out, in_=sc[:, 0:2]).then_inc(out_sem, 16)
            sync.wait_ge(out_sem, 16)

    return nc


def make_in_maps(x_env: np.ndarray, y_env: np.ndarray) -> list[dict[str, np.ndarray]]:
    x = np.asarray(x_env, dtype=np.float32).reshape(ROWS, T)
    y = np.asarray(y_env, dtype=np.float32).reshape(ROWS, T)
    in_maps = []
    for i in range(N_CORES):
        shard = np.concatenate(
            [x[i * RPC : (i + 1) * RPC], y[i * RPC : (i + 1) * RPC]], axis=0
        )
        in_maps.append({"xy": np.ascontiguousarray(shard)})
    return in_maps


def lufs_from_nd(nd: np.ndarray) -> np.ndarray:
    """Per-row LUFS from the device's [128, 2] (numar', denar) output."""
    nd = np.asarray(nd, dtype=np.float64).reshape(128, 2)
    z_ave_ar = nd[:, 0] / FRAME / (nd[:, 1] + EPS)
    return -0.691 + 10.0 * np.log10(z_ave_ar + EPS)


def finish(per_core_nd: list[np.ndarray]) -> np.ndarray:
    total = 0.0
    for nd in per_core_nd:
        lf = lufs_from_nd(nd)
        total += np.maximum(lf[RPC:] - lf[:RPC], 0.0).sum()
    return np.array(ALPHA * total, dtype=np.float32)


def kernel(x_env: np.ndarray, y_env: np.ndarray) -> np.ndarray:
    nc = _build_program()
    in_maps = make_in_maps(x_env, y_env)
    res = run_bass_kernel_spmd(nc, in_maps, core_ids=list(range(N_CORES)))
    return finish([res.results[i]["nd"] for i in range(N_CORES)])


# revision 3
# speedup vs baseline: 2.5752x; 1.0106x over previous
"""DHASPI level-loss kernel for 8 Trainium2 NeuronCores.

Data-parallel over the fused B*C row axis: each of the 8 cores processes 64
rows of x_env and 64 rows of y_env (x rows in SBUF partitions 0-63, y rows in
64-127, so every DMA is a full 128-partition transfer).

Dataflow per core:
- gpsimd (SWDGE) cast-DMA streams chunks HBM f32 -> SBUF fp8(e4m3).
  fp8 is safe here: the loss compares 10*log10 of ~192000-sample mean-square
  energies between x and y; e4m3's quantization bias on E[x^2] (~+0.13%) is
  identical for x and y rows so it cancels in the loudness difference, and
  the per-row random part is ~0.005 dB against a 2e-2 loss tolerance
  (measured: loss rel err ~3e-3).
- Each 960-sample block is squared-and-summed into one f32 column of the
  block-sum tile bs[128, 200] by one of the two elementwise engines:
    ACT:  activation(Square, accum_out=bs column)   (~1.17 us/block)
    DVE:  scalar_tensor_tensor(x*1*x, accum_out)    (~1.06 us/block)
  94 blocks go to ACT and 106 to DVE so both engines carry ~110 us per core
  and finish together. The first chunk is split [2, 8] blocks so the engines
  start ~2.7 us sooner.
- bs is DMA'd out (800 B/partition); the host does the cheap tail exactly as
  the reference (float64): overlapped frame energies (frame f = blocks
  3f..3f+9), absolute + relative gating, log10, and the relu-diff scalar sum.
  This removes the serialized device epilogue and any activation-table load.

Raw Bass (explicit semaphores): the Tile framework's kernel-tail drain emits
multi-wait instructions this walrus build rejects. tensor_tensor_reduce is
rejected by walrus codegen (visitInstISA), but activation/stt accum_out both
compile and are bit-exact on HW vs numpy (probed).
"""

import numpy as np

import concourse.bass as bass
from concourse import mybir
from concourse.bass_utils import run_bass_kernel_spmd

# Problem constants (hardcoded from the spec; kernel.py must be self-contained)
B, C, T = 16, 32, 192000
N_CORES = 8
ROWS = B * C  # 512
RPC = ROWS // N_CORES  # 64 rows per core per tensor

FRAME = 9600
SHIFT = 2880
BLK = 960  # gcd(FRAME, SHIFT)
NBLK = T // BLK  # 200 block sums per row
NFRM = (T - FRAME) // SHIFT + 1  # 64 frames per row

# Chunking in 960-sample blocks: first chunk split small so the engines start
# early; slots are sized for the largest chunk (10 blocks).
CHUNK_BLOCKS = [2, 8] + [10] * 19  # 21 chunks, 200 blocks
N_CHUNKS = len(CHUNK_BLOCKS)
MAX_CHUNK = 10 * BLK
NBUF = 4  # input chunk slots

# ACT blocks per chunk (leading blocks; the rest go to DVE). Totals: ACT 94,
# DVE 106 -> ACT ~110.2 us, DVE ~109.5 us per core, balanced.
_act_split = {0: 1, 1: 4}
_fours = {4, 7, 10, 14, 17, 20}  # six 10-block chunks get 4 ACT blocks
ACT_BLOCKS = [
    _act_split.get(c, 4 if c in _fours else 5) for c in range(N_CHUNKS)
]
assert sum(ACT_BLOCKS) == 94

EPS = 1e-8
ALPHA = 1e-4
GAMMA_A = -70.0

F32 = mybir.dt.float32
BF16 = mybir.dt.bfloat16
FP8 = mybir.dt.float8e4


def _build_program() -> bass.Bass:
    nc = bass.Bass("TRN2", target_bir_lowering=False, debug=False)
    AF = mybir.ActivationFunctionType
    ALU = mybir.AluOpType

    xy = nc.dram_tensor("xy", [128, T], F32, kind="ExternalInput").ap()
    out = nc.dram_tensor("bs_out", [128, NBLK], F32, kind="ExternalOutput").ap()

    xt = [
        nc.alloc_sbuf_tensor(f"xt{i}", [128, MAX_CHUNK], FP8).ap()
        for i in range(NBUF)
    ]
    junk_a = nc.alloc_sbuf_tensor("junk_a", [128, BLK], BF16).ap()
    junk_d = nc.alloc_sbuf_tensor("junk_d", [128, BLK], BF16).ap()
    bs = nc.alloc_sbuf_tensor("bs", [128, NBLK], F32).ap()

    # chunk start offsets in blocks, and per-slot use counters
    starts = np.cumsum([0] + CHUNK_BLOCKS[:-1]).tolist()
    slot_use = [0] * NBUF
    use_idx = []  # per chunk: how many times its slot was used before
    for c in range(N_CHUNKS):
        s = c % NBUF
        use_idx.append(slot_use[s])
        slot_use[s] += 1

    with (
        nc.Block() as block,
        nc.semaphore("dma_sem0") as dma_sem0,
        nc.semaphore("dma_sem1") as dma_sem1,
        nc.semaphore("dma_sem2") as dma_sem2,
        nc.semaphore("dma_sem3") as dma_sem3,
        nc.semaphore("act_sem") as act_sem,
        nc.semaphore("dve_sem") as dve_sem,
        nc.semaphore("out_sem") as out_sem,
    ):
        # One DMA-completion sem per buffer slot: a shared cumulative sem is
        # unsafe with >1 DMA in flight (per-engine increments of a later DMA
        # can reach the threshold while an earlier one is still draining).
        dma_sems = [dma_sem0, dma_sem1, dma_sem2, dma_sem3]

        @block.gpsimd
        def _(g):
            for c in range(N_CHUNKS):
                if c >= NBUF:
                    # slot c%NBUF is free once both engines finished c-NBUF
                    g.wait_ge(act_sem, c - NBUF + 1)
                    g.wait_ge(dve_sem, c - NBUF + 1)
                off = starts[c] * BLK
                size = CHUNK_BLOCKS[c] * BLK
                g.dma_start(
                    out=xt[c % NBUF][:, 0:size], in_=xy[:, off : off + size]
                ).then_inc(dma_sems[c % NBUF], 16)

        @block.scalar
        def _(scalar):
            for c in range(N_CHUNKS):
                scalar.wait_ge(dma_sems[c % NBUF], (use_idx[c] + 1) * 16)
                for b in range(ACT_BLOCKS[c]):
                    col = starts[c] + b
                    scalar.activation(
                        junk_a,
                        xt[c % NBUF][:, b * BLK : (b + 1) * BLK],
                        AF.Square,
                        accum_out=bs[:, col : col + 1],
                    )
                # flush writes before signalling readers on other queues
                scalar.drain().then_inc(act_sem, 1)

        @block.vector
        def _(v):
            for c in range(N_CHUNKS):
                v.wait_ge(dma_sems[c % NBUF], (use_idx[c] + 1) * 16)
                for b in range(ACT_BLOCKS[c], CHUNK_BLOCKS[c]):
                    col = starts[c] + b
                    blk_ap = xt[c % NBUF][:, b * BLK : (b + 1) * BLK]
                    v.scalar_tensor_tensor(
                        out=junk_d,
                        in0=blk_ap,
                        scalar=1.0,
                        in1=blk_ap,
                        op0=mybir.AluOpType.mult,
                        op1=mybir.AluOpType.mult,
                        accum_out=bs[:, col : col + 1],
                    )
                v.drain().then_inc(dve_sem, 1)

        @block.sync
        def _(sync):
            sync.wait_ge(act_sem, N_CHUNKS)
            sync.wait_ge(dve_sem, N_CHUNKS)
            sync.dma_start(out=out, in_=bs).then_inc(out_sem, 16)
            sync.wait_ge(out_sem, 16)

    return nc


def make_in_maps(x_env: np.ndarray, y_env: np.ndarray) -> list[dict[str, np.ndarray]]:
    x = np.asarray(x_env, dtype=np.float32).reshape(ROWS, T)
    y = np.asarray(y_env, dtype=np.float32).reshape(ROWS, T)
    in_maps = []
    for i in range(N_CORES):
        shard = np.concatenate(
            [x[i * RPC : (i + 1) * RPC], y[i * RPC : (i + 1) * RPC]], axis=0
        )
        in_maps.append({"xy": np.ascontiguousarray(shard)})
    return in_maps


def lufs_from_bs(bs: np.ndarray) -> np.ndarray:
    """Per-row LUFS from the device's [128, NBLK] f32 block energy sums.

    Mirrors reference.measure_loudness in float64: frame f = blocks 3f..3f+9,
    z = frame_sum / FRAME, then absolute and relative gating.
    """
    bs = np.asarray(bs, dtype=np.float64).reshape(128, NBLK)
    # overlapped frame sums: [128, NFRM]
    idx = 3 * np.arange(NFRM)[:, None] + np.arange(FRAME // BLK)[None, :]
    z = bs[:, idx].sum(axis=2) / FRAME
    el = -0.691 + 10.0 * np.log10(z + EPS)
    idx_a = (el > GAMMA_A).astype(np.float64)
    z_ave_a = (z * idx_a).sum(1) / (idx_a.sum(1) + EPS)
    gamma_r = -0.691 + 10.0 * np.log10(z_ave_a + EPS) - 10.0
    idx_ar = idx_a * (el > gamma_r[:, None])
    z_ave_ar = (z * idx_ar).sum(1) / (idx_ar.sum(1) + EPS)
    return -0.691 + 10.0 * np.log10(z_ave_ar + EPS)


def finish(per_core_bs: list[np.ndarray]) -> np.ndarray:
    total = 0.0
    for bsv in per_core_bs:
        lf = lufs_from_bs(bsv)
        total += np.maximum(lf[RPC:] - lf[:RPC], 0.0).sum()
    return np.array(ALPHA * total, dtype=np.float32)


def kernel(x_env: np.ndarray, y_env: np.ndarray) -> np.ndarray:
    nc = _build_program()
    in_maps = make_in_maps(x_env, y_env)
    res = run_bass_kernel_spmd(nc, in_maps, core_ids=list(range(N_CORES)))
    return finish([res.results[i]["bs_out"] for i in range(N_CORES)])


# revision 5
# speedup vs baseline: 2.6643x; 1.0346x over previous
"""DHASPI level-loss kernel for 8 Trainium2 NeuronCores.

Data-parallel over the fused B*C row axis: each of the 8 cores processes 64
rows of x_env and 64 rows of y_env (x rows in SBUF partitions 0-63, y rows in
64-127, so every DMA is a full 128-partition transfer).

Dataflow per core:
- gpsimd (SWDGE) cast-DMA streams chunks HBM f32 -> SBUF fp8(e4m3).
  fp8 is safe here: the loss compares 10*log10 of ~192000-sample mean-square
  energies between x and y; e4m3's quantization bias on E[x^2] (~+0.13%) is
  identical for x and y rows so it cancels in the loudness difference, and
  the per-row random part is ~0.005 dB against a 2e-2 loss tolerance
  (measured: loss rel err ~3e-3).
- Each 960-sample block is squared-and-summed into one f32 column of the
  block-sum tile bs[128, 200] by one of the two elementwise engines:
    ACT:  activation(Square, accum_out=bs column)   (~1.17 us/block)
    DVE:  scalar_tensor_tensor(x*1*x, accum_out)    (~1.06 us/block)
  94 blocks go to ACT and 106 to DVE so both engines carry ~110 us per core
  and finish together. The first chunk is split [2, 8] blocks so the engines
  start ~2.7 us sooner.
- bs is DMA'd out (800 B/partition); the host does the cheap tail exactly as
  the reference (float64): overlapped frame energies (frame f = blocks
  3f..3f+9), absolute + relative gating, log10, and the relu-diff scalar sum.
  This removes the serialized device epilogue and any activation-table load.

Raw Bass (explicit semaphores): the Tile framework's kernel-tail drain emits
multi-wait instructions this walrus build rejects. tensor_tensor_reduce is
rejected by walrus codegen (visitInstISA), but activation/stt accum_out both
compile and are bit-exact on HW vs numpy (probed).
"""

import numpy as np

import concourse.bass as bass
from concourse import mybir
from concourse.bass_utils import run_bass_kernel_spmd

# Problem constants (hardcoded from the spec; kernel.py must be self-contained)
B, C, T = 16, 32, 192000
N_CORES = 8
ROWS = B * C  # 512
RPC = ROWS // N_CORES  # 64 rows per core per tensor

FRAME = 9600
SHIFT = 2880
BLK = 960  # gcd(FRAME, SHIFT)
NBLK = T // BLK  # 200 block sums per row
NFRM = (T - FRAME) // SHIFT + 1  # 64 frames per row

# Chunking in 960-sample blocks: small leading chunks so the engines start
# ~2.7 us sooner, a small final chunk so they finish together; slots are
# sized for the largest chunk (10 blocks).
CHUNK_BLOCKS = [2, 2, 4, 8] + [10] * 18 + [4]  # 23 chunks, 200 blocks
N_CHUNKS = len(CHUNK_BLOCKS)
MAX_CHUNK = 10 * BLK
NBUF = 4  # input chunk slots

# ACT blocks per chunk (leading blocks; the rest go to DVE). Totals: ACT 95,
# DVE 105 -> both engines carry ~111.3 us per core (ACT ~1.172 us/block,
# DVE ~1.060 us/block, measured in the timeline sim).
_act_split = {0: 1, 1: 1, 2: 2, 3: 4, 22: 2}
_fours = {6, 10, 14, 18, 21}  # five 10-block chunks get 4 ACT blocks
ACT_BLOCKS = [
    _act_split.get(c, 4 if c in _fours else 5) for c in range(N_CHUNKS)
]
assert sum(ACT_BLOCKS) == 95, sum(ACT_BLOCKS)

EPS = 1e-8
ALPHA = 1e-4
GAMMA_A = -70.0

F32 = mybir.dt.float32
BF16 = mybir.dt.bfloat16
FP8 = mybir.dt.float8e4


def _build_program() -> bass.Bass:
    nc = bass.Bass("TRN2", target_bir_lowering=False, debug=False)
    AF = mybir.ActivationFunctionType
    ALU = mybir.AluOpType

    xy = nc.dram_tensor("xy", [128, T], F32, kind="ExternalInput").ap()
    out = nc.dram_tensor("bs_out", [128, NBLK], F32, kind="ExternalOutput").ap()

    xt = [
        nc.alloc_sbuf_tensor(f"xt{i}", [128, MAX_CHUNK], FP8).ap()
        for i in range(NBUF)
    ]
    junk_a = nc.alloc_sbuf_tensor("junk_a", [128, BLK], BF16).ap()
    junk_d = nc.alloc_sbuf_tensor("junk_d", [128, BLK], BF16).ap()
    bs = nc.alloc_sbuf_tensor("bs", [128, NBLK], F32).ap()

    # chunk start offsets in blocks, and per-slot use counters
    starts = np.cumsum([0] + CHUNK_BLOCKS[:-1]).tolist()
    slot_use = [0] * NBUF
    use_idx = []  # per chunk: how many times its slot was used before
    for c in range(N_CHUNKS):
        s = c % NBUF
        use_idx.append(slot_use[s])
        slot_use[s] += 1

    with (
        nc.Block() as block,
        nc.semaphore("dma_sem0") as dma_sem0,
        nc.semaphore("dma_sem1") as dma_sem1,
        nc.semaphore("dma_sem2") as dma_sem2,
        nc.semaphore("dma_sem3") as dma_sem3,
        nc.semaphore("act_sem") as act_sem,
        nc.semaphore("dve_sem") as dve_sem,
        nc.semaphore("out_sem") as out_sem,
    ):
        # One DMA-completion sem per buffer slot: a shared cumulative sem is
        # unsafe with >1 DMA in flight (per-engine increments of a later DMA
        # can reach the threshold while an earlier one is still draining).
        dma_sems = [dma_sem0, dma_sem1, dma_sem2, dma_sem3]

        @block.gpsimd
        def _(g):
            for c in range(N_CHUNKS):
                if c >= NBUF:
                    # slot c%NBUF is free once both engines finished c-NBUF
                    g.wait_ge(act_sem, c - NBUF + 1)
                    g.wait_ge(dve_sem, c - NBUF + 1)
                off = starts[c] * BLK
                size = CHUNK_BLOCKS[c] * BLK
                g.dma_start(
                    out=xt[c % NBUF][:, 0:size], in_=xy[:, off : off + size]
                ).then_inc(dma_sems[c % NBUF], 16)

        @block.scalar
        def _(scalar):
            for c in range(N_CHUNKS):
                scalar.wait_ge(dma_sems[c % NBUF], (use_idx[c] + 1) * 16)
                for b in range(ACT_BLOCKS[c]):
                    col = starts[c] + b
                    scalar.activation(
                        junk_a,
                        xt[c % NBUF][:, b * BLK : (b + 1) * BLK],
                        AF.Square,
                        accum_out=bs[:, col : col + 1],
                    )
                # flush writes before signalling readers on other queues
                scalar.drain().then_inc(act_sem, 1)

        @block.vector
        def _(v):
            for c in range(N_CHUNKS):
                v.wait_ge(dma_sems[c % NBUF], (use_idx[c] + 1) * 16)
                for b in range(ACT_BLOCKS[c], CHUNK_BLOCKS[c]):
                    col = starts[c] + b
                    blk_ap = xt[c % NBUF][:, b * BLK : (b + 1) * BLK]
                    v.scalar_tensor_tensor(
                        out=junk_d,
                        in0=blk_ap,
                        scalar=1.0,
                        in1=blk_ap,
                        op0=mybir.AluOpType.mult,
                        op1=mybir.AluOpType.mult,
                        accum_out=bs[:, col : col + 1],
                    )
                v.drain().then_inc(dve_sem, 1)

        @block.sync
        def _(sync):
            sync.wait_ge(act_sem, N_CHUNKS)
            sync.wait_ge(dve_sem, N_CHUNKS)
            sync.dma_start(out=out, in_=bs).then_inc(out_sem, 16)
            sync.wait_ge(out_sem, 16)

    return nc


def make_in_maps(x_env: np.ndarray, y_env: np.ndarray) -> list[dict[str, np.ndarray]]:
    x = np.asarray(x_env, dtype=np.float32).reshape(ROWS, T)
    y = np.asarray(y_env, dtype=np.float32).reshape(ROWS, T)
    in_maps = []
    for i in range(N_CORES):
        shard = np.concatenate(
            [x[i * RPC : (i + 1) * RPC], y[i * RPC : (i + 1) * RPC]], axis=0
        )
        in_maps.append({"xy": np.ascontiguousarray(shard)})
    return in_maps


def lufs_from_bs(bs: np.ndarray) -> np.ndarray:
    """Per-row LUFS from the device's [128, NBLK] f32 block energy sums.

    Mirrors reference.measure_loudness in float64: frame f = blocks 3f..3f+9,
    z = frame_sum / FRAME, then absolute and relative gating.
    """
    bs = np.asarray(bs, dtype=np.float64).reshape(128, NBLK)
    # overlapped frame sums: [128, NFRM]
    idx = 3 * np.arange(NFRM)[:, None] + np.arange(FRAME // BLK)[None, :]
    z = bs[:, idx].sum(axis=2) / FRAME
    el = -0.691 + 10.0 * np.log10(z + EPS)
    idx_a = (el > GAMMA_A).astype(np.float64)
    z_ave_a = (z * idx_a).sum(1) / (idx_a.sum(1) + EPS)
    gamma_r = -0.691 + 10.0 * np.log10(z_ave_a + EPS) - 10.0
    idx_ar = idx_a * (el > gamma_r[:, None])
    z_ave_ar = (z * idx_ar).sum(1) / (idx_ar.sum(1) + EPS)
    return -0.691 + 10.0 * np.log10(z_ave_ar + EPS)


def finish(per_core_bs: list[np.ndarray]) -> np.ndarray:
    total = 0.0
    for bsv in per_core_bs:
        lf = lufs_from_bs(bsv)
        total += np.maximum(lf[RPC:] - lf[:RPC], 0.0).sum()
    return np.array(ALPHA * total, dtype=np.float32)


def kernel(x_env: np.ndarray, y_env: np.ndarray) -> np.ndarray:
    nc = _build_program()
    in_maps = make_in_maps(x_env, y_env)
    res = run_bass_kernel_spmd(nc, in_maps, core_ids=list(range(N_CORES)))
    return finish([res.results[i]["bs_out"] for i in range(N_CORES)])
